# revision 16
# baseline (speedup 1.0000x reference)
"""Distributed multi-head attention (RoPE) kernel for 8 TRN2 NeuronCores.

Sharding: tensor-parallel over heads. 16 heads / 8 cores = 2 heads per core.
Each core projects q/k/v for its 2 heads (full sequence), runs attention,
then an AllToAll converts head-sharding -> token-sharding so each core
applies the full Wo to its 512-token shard. Output is token-sharded
[H, 512] per core (transposed); host reassembles.

Matmul operands are bf16 (host-cast inputs); all accumulation, softmax
and RoPE arithmetic stays fp32 (PSUM accumulate + fp32 cos/sin).

Layout notes (device tensors are feature-major / transposed):
  hiddenT [H=2048, B*S=4096]     (host pre-transposed, replicated)
  qT/kT per head [HD=128, 4096]  from projection, RoPE applied on eviction
  v natural [4096, 256]          (both heads side by side)
  scoresT [j,i] tiles -> exp -> PT; out_hT[d,i] = sum_j v[j,d]*PT[j,i]
  row sums via ones-vector matmul; division via K=1 broadcast matmul.
  yT [H, 512] = WoT-chunks^T @ outT_gathered.
"""

import sys

sys.path.insert(0, "/opt/trn_rl_repo")

from contextlib import ExitStack

import ml_dtypes
import numpy as np

import concourse.bass as bass
import concourse.tile as tile
from concourse import bacc, mybir
from concourse.bass_utils import run_bass_kernel_spmd

F32 = mybir.dt.float32
BF16 = mybir.dt.bfloat16
NPBF = ml_dtypes.bfloat16

B, S, H = 2, 2048, 2048
NH, HD = 16, 128
NCORES = 8
NH_LOC = NH // NCORES          # 2 heads per core
T = B * S                      # 4096 tokens
TT = 512                       # token tile
NT = T // TT                   # 8 token tiles
KC = H // 128                  # 16 contraction chunks
SHARD = T // NCORES            # 512 tokens per core output shard
INV_SQRT_D = 1.0 / float(np.sqrt(HD))

_CACHE = {}


def build_graph():
    nc = bacc.Bacc("TRN2", target_bir_lowering=False, debug=False,
                   num_devices=NCORES)

    hiddenT = nc.dram_tensor("hiddenT", [H, T], BF16, kind="ExternalInput")
    cosT = nc.dram_tensor("cosT", [HD, S], F32, kind="ExternalInput")
    nsinT = nc.dram_tensor("nsinT", [HD, S], F32, kind="ExternalInput")
    wqT = nc.dram_tensor("wqT", [H, NH_LOC * HD], BF16, kind="ExternalInput")
    wkT = nc.dram_tensor("wkT", [H, NH_LOC * HD], BF16, kind="ExternalInput")
    wvT = nc.dram_tensor("wvT", [H, NH_LOC * HD], BF16, kind="ExternalInput")
    woT = nc.dram_tensor("woT", [H, H], BF16, kind="ExternalInput")
    out = nc.dram_tensor("out", [H, SHARD], F32, kind="ExternalOutput")

    with tile.TileContext(nc) as tc:
        with ExitStack() as big:
            const = big.enter_context(tc.tile_pool(name="const", bufs=1))
            ones_k = const.tile([128, 1], BF16, tag="ones_k")
            nc.any.memset(ones_k[:], 1.0)
            ones_m = const.tile([1, 128], BF16, tag="ones_m")
            nc.any.memset(ones_m[:], 1.0)

            cs_pool = big.enter_context(tc.tile_pool(name="cs", bufs=1))
            cos_t = cs_pool.tile([HD, S], F32, tag="cos")
            nsin_t = cs_pool.tile([HD, S], F32, tag="nsin")

            # WoT prefetch pool: created before act_stack so pool stack
            # order holds; resident before the collectives run, so its DMA
            # traffic never contends with the AllToAlls or the final matmuls
            wop = big.enter_context(tc.tile_pool(name="wop", bufs=4 * KC))
            wo_t = {}
            for gg in range(4):
                for f in range(KC):
                    t = wop.tile([128, 512], BF16, tag="wo",
                                 name=f"wo{gg}_{f}")
                    nc.gpsimd.dma_start(
                        t[:], woT[128 * f:128 * (f + 1),
                                  512 * gg:512 * (gg + 1)])
                    wo_t[(gg, f)] = t

            # ---- long-lived activation pools (freed before final proj) ----
            act_stack = ExitStack()
            wpool = act_stack.enter_context(tc.tile_pool(name="w", bufs=3 * KC))
            w_t = {}
            for name, src in (("q", wqT), ("k", wkT), ("v", wvT)):
                for f in range(KC):
                    t = wpool.tile([128, NH_LOC * HD], BF16, tag="w",
                                   name=f"w_{name}_{f}")
                    eng = nc.sync if name == "q" else nc.gpsimd
                    eng.dma_start(t[:], src[128 * f:128 * (f + 1), :])
                    w_t[(name, f)] = t
            nc.gpsimd.dma_start(cos_t[:], cosT[:, :])
            nc.gpsimd.dma_start(nsin_t[:], nsinT[:, :])

            qk_pool = act_stack.enter_context(tc.tile_pool(name="qk", bufs=4 * NT))
            v_pool = act_stack.enter_context(tc.tile_pool(name="v", bufs=T // 128))

            qk_t = {}   # (qk, head, ttile) -> [128, TT] sbuf bf16
            v_t = []    # t-chunk -> [128, NH_LOC*HD] sbuf bf16

            # ---------------- phase 1: projections + RoPE ----------------
            with ExitStack() as ph1:
                ht_pool = ph1.enter_context(tc.tile_pool(name="ht", bufs=KC + 2))
                psqk = ph1.enter_context(
                    tc.tile_pool(name="psqk", bufs=5, space="PSUM"))
                psv = ph1.enter_context(
                    tc.tile_pool(name="psv", bufs=3, space="PSUM"))
                tqp = ph1.enter_context(tc.tile_pool(name="tqp", bufs=2))
                rotp = ph1.enter_context(tc.tile_pool(name="rotp", bufs=2))

                for tt in range(NT):
                    t0 = tt * TT
                    i0 = t0 % S  # position within batch (cos/sin index)
                    ht = [ht_pool.tile([128, TT], BF16, tag="ht",
                                       name=f"ht{tt}_{i}")
                          for i in range(KC)]
                    for f in range(KC):
                        nc.sync.dma_start(
                            ht[f][:], hiddenT[128 * f:128 * (f + 1),
                                              t0:t0 + TT])
                    # q/k projections per head -> PSUM [128=HD, TT]
                    for name in ("q", "k"):
                        for h in range(NH_LOC):
                            ps = psqk.tile([128, TT], F32, tag="psqk")
                            for f in range(KC):
                                nc.tensor.matmul(
                                    ps[:],
                                    w_t[(name, f)][:, 128 * h:128 * (h + 1)],
                                    ht[f][:],
                                    start=(f == 0), stop=(f == KC - 1))
                            # RoPE: rot = shifted halves * nsin; out = x*cos + rot
                            tq = tqp.tile([128, TT], F32, tag="tq")
                            nc.vector.scalar_tensor_tensor(
                                tq[:], ps[:], 0.0, cos_t[:, i0:i0 + TT],
                                op0=mybir.AluOpType.bypass,
                                op1=mybir.AluOpType.mult)
                            rot = rotp.tile([128, TT], F32, tag="rot")
                            nc.vector.scalar_tensor_tensor(
                                rot[0:64, :], ps[64:128, :], 0.0,
                                nsin_t[0:64, i0:i0 + TT],
                                op0=mybir.AluOpType.bypass,
                                op1=mybir.AluOpType.mult)
                            nc.vector.scalar_tensor_tensor(
                                rot[64:128, :], ps[0:64, :], 0.0,
                                nsin_t[64:128, i0:i0 + TT],
                                op0=mybir.AluOpType.bypass,
                                op1=mybir.AluOpType.mult)
                            dst = qk_pool.tile([128, TT], BF16, tag="qk")
                            nc.vector.scalar_tensor_tensor(
                                dst[:], tq[:], 0.0, rot[:],
                                op0=mybir.AluOpType.bypass,
                                op1=mybir.AluOpType.add)
                            qk_t[(name, h, tt)] = dst
                    # v natural layout: [t128, 256] both heads
                    for sub in range(TT // 128):
                        ps = psv.tile([128, NH_LOC * HD], F32, tag="psv")
                        for f in range(KC):
                            nc.tensor.matmul(
                                ps[:],
                                ht[f][:, 128 * sub:128 * (sub + 1)],
                                w_t[("v", f)][:],
                                start=(f == 0), stop=(f == KC - 1))
                        vt = v_pool.tile([128, NH_LOC * HD], BF16, tag="v")
                        nc.scalar.copy(vt[:], ps[:])
                        v_t.append(vt)

            # --------- phase 2: attention + A2A bounce-in writes ---------
            dram = big.enter_context(tc.tile_pool(name="dram", bufs=1,
                                                  space="DRAM"))
            bi_h = [dram.tile([NCORES * 128, SHARD], BF16, tag=f"bi{h}",
                              name=f"bi{h}") for h in range(NH_LOC)]
            bo_h = [dram.tile([NCORES * 128, SHARD], BF16, tag=f"bo{h}",
                              name=f"bo{h}") for h in range(NH_LOC)]

            with ExitStack() as ph2:
                stp = ph2.enter_context(
                    tc.tile_pool(name="stp", bufs=5, space="PSUM"))
                accp = ph2.enter_context(
                    tc.tile_pool(name="accp", bufs=2, space="PSUM"))
                rsp = ph2.enter_context(
                    tc.tile_pool(name="rsp", bufs=1, space="PSUM"))
                ptp = ph2.enter_context(tc.tile_pool(name="ptp", bufs=4))
                smallp = ph2.enter_context(tc.tile_pool(name="smallp", bufs=4))
                rbsb = ph2.enter_context(tc.tile_pool(name="rbsb", bufs=2))
                ofp = ph2.enter_context(tc.tile_pool(name="ofp", bufs=3))

                for h in range(NH_LOC):
                    for b in range(B):
                        for ib in range(S // TT):    # i-block within batch
                            q_tile = qk_t[("q", h, 4 * b + ib)]
                            acc = accp.tile([128, TT], F32, tag="accp",
                                            name="acc")
                            rs = rsp.tile([1, TT], F32, tag="rsp", name="rs")
                            sts = [None] * 16

                            def emit_st(j):
                                kt = qk_t[("k", h, 4 * b + j // 4)]
                                co = 128 * (j % 4)
                                sts[j] = stp.tile([128, TT], F32, tag="stp",
                                                  name="st")
                                nc.tensor.matmul(
                                    sts[j][:], kt[:, co:co + 128],
                                    q_tile[:], start=True, stop=True)

                            emit_st(0)
                            emit_st(1)
                            emit_st(2)
                            for jt in range(S // 128):   # j chunks of 128
                                if jt + 3 < 16:
                                    emit_st(jt + 3)
                                pt = ptp.tile([128, TT], BF16, tag="ptp",
                                              name="pt")
                                nc.scalar.activation(
                                    pt[:], sts[jt][:],
                                    mybir.ActivationFunctionType.Exp,
                                    scale=INV_SQRT_D)
                                nc.tensor.matmul(
                                    acc[:],
                                    v_t[16 * b + jt][:, 128 * h:128 * (h + 1)],
                                    pt[:],
                                    start=(jt == 0), stop=(jt == 15))
                                nc.tensor.matmul(
                                    rs[:], ones_k[:], pt[:],
                                    start=(jt == 0), stop=(jt == 15))
                            # softmax epilogue: no PE ops (gpsimd bcast)
                            rs_sb = smallp.tile([1, TT], F32, tag="rs_sb",
                                                name="rs_sb")
                            nc.scalar.copy(rs_sb[:], rs[:])
                            rec = smallp.tile([1, TT], F32, tag="rec",
                                              name="rec")
                            nc.vector.reciprocal_approx_fast(rec[:], rs_sb[:])
                            rsb = rbsb.tile([128, TT], F32, tag="rb_sb",
                                            name="rsb")
                            nc.gpsimd.partition_broadcast(rsb[:], rec[:])
                            of = ofp.tile([128, TT], BF16, tag="of", name="of")
                            nc.vector.scalar_tensor_tensor(
                                of[:], acc[:], 0.0, rsb[:],
                                op0=mybir.AluOpType.bypass,
                                op1=mybir.AluOpType.mult)
                            row0 = 128 * (4 * b + ib)
                            nc.sync.dma_start(
                                bi_h[h][row0:row0 + 128, :], of[:])
                    if h == 0:
                        nc.gpsimd.collective_compute(
                            "AllToAll", mybir.AluOpType.bypass,
                            replica_groups=[list(range(NCORES))],
                            ins=[bi_h[0][:].opt()], outs=[bo_h[0][:].opt()])

            act_stack.close()   # free qk/v/w SBUF before final phase

            # ---------- phase 3: second AllToAll + output projection ----------
            nc.gpsimd.collective_compute(
                "AllToAll", mybir.AluOpType.bypass,
                replica_groups=[list(range(NCORES))],
                ins=[bi_h[1][:].opt()], outs=[bo_h[1][:].opt()])

            with ExitStack() as ph3:
                ogp = ph3.enter_context(tc.tile_pool(name="ogp", bufs=KC))
                yps = ph3.enter_context(
                    tc.tile_pool(name="yps", bufs=4, space="PSUM"))
                ysb = ph3.enter_context(tc.tile_pool(name="ysb", bufs=4))

                og = []
                for f in range(KC):     # f = 2r + h -> bo_h[h] rows 128r
                    t = ogp.tile([128, SHARD], BF16, tag="og", name=f"og{f}")
                    r, hh = f // 2, f % 2
                    nc.sync.dma_start(
                        t[:], bo_h[hh][128 * r:128 * (r + 1), :])
                    og.append(t)
                for gg in range(4):              # groups of 4 g-tiles
                    wo_g = [wo_t[(gg, f)] for f in range(KC)]
                    for gi in range(4):
                        g = 4 * gg + gi
                        yp = yps.tile([128, SHARD], F32, tag="yps", name="yp")
                        for fi, f in enumerate(
                                [x for x in range(KC) if x % 2 == 0]
                                + [x for x in range(KC) if x % 2 == 1]):
                            nc.tensor.matmul(
                                yp[:],
                                wo_g[f][:, 128 * gi:128 * (gi + 1)],
                                og[f][:],
                                start=(fi == 0), stop=(fi == KC - 1))
                        ys = ysb.tile([128, SHARD], F32, tag="ysb", name="ys")
                        nc.scalar.copy(ys[:], yp[:])
                        nc.sync.dma_start(out[128 * g:128 * (g + 1), :], ys[:])

    nc.compile()
    return nc


def _prep_inputs(hidden, cos, sin, Wq, Wk, Wv, Wo):
    hf = np.ascontiguousarray(hidden.reshape(T, H).T.astype(NPBF))
    cosT = np.ascontiguousarray(cos.T).astype(np.float32)
    nsinT = np.ascontiguousarray(sin.T).astype(np.float32)
    nsinT[0:HD // 2] *= -1.0
    woT = np.ascontiguousarray(Wo.T.astype(NPBF))
    in_maps = []
    for c in range(NCORES):
        r0, r1 = 256 * c, 256 * (c + 1)
        in_maps.append({
            "hiddenT": hf,
            "cosT": cosT,
            "nsinT": nsinT,
            "wqT": np.ascontiguousarray(Wq[r0:r1].T.astype(NPBF)),
            "wkT": np.ascontiguousarray(Wk[r0:r1].T.astype(NPBF)),
            "wvT": np.ascontiguousarray(Wv[r0:r1].T.astype(NPBF)),
            "woT": woT,
        })
    return in_maps


def kernel(hidden, cos, sin, attention_mask, Wq, Wk, Wv, Wo, **run_kwargs):
    if "nc" not in _CACHE:
        _CACHE["nc"] = build_graph()
    nc = _CACHE["nc"]
    in_maps = _prep_inputs(hidden, cos, sin, Wq, Wk, Wv, Wo)
    res = run_bass_kernel_spmd(nc, in_maps, core_ids=list(range(NCORES)),
                               **run_kwargs)
    _CACHE["last_result"] = res
    outs = res.results if hasattr(res, "results") else res
    y = np.empty((T, H), dtype=np.float32)
    for c in range(NCORES):
        y[SHARD * c:SHARD * (c + 1), :] = outs[c]["out"].T
    return y.reshape(B, S, H)


# revision 17
# speedup vs baseline: 1.0424x; 1.0424x over previous
"""Distributed multi-head attention (RoPE) kernel for 8 TRN2 NeuronCores.

Sharding: tensor-parallel over heads. 16 heads / 8 cores = 2 heads per core.
Each core projects q/k/v for its 2 heads (full sequence), runs attention,
then an AllToAll converts head-sharding -> token-sharding so each core
applies the full Wo to its 512-token shard. Output is token-sharded
[H, 512] per core (transposed); host reassembles.

Matmul operands are bf16 (host-cast inputs); all accumulation, softmax
and RoPE arithmetic stays fp32 (PSUM accumulate + fp32 cos/sin).

Layout notes (device tensors are feature-major / transposed):
  hiddenT [H=2048, B*S=4096]     (host pre-transposed, replicated)
  qT/kT per head [HD=128, 4096]  from projection, RoPE applied on eviction
  v natural [4096, 256]          (both heads side by side)
  scoresT [j,i] tiles -> exp -> PT; out_hT[d,i] = sum_j v[j,d]*PT[j,i]
  row sums via ones-vector matmul; division via K=1 broadcast matmul.
  yT [H, 512] = WoT-chunks^T @ outT_gathered.
"""

import sys

sys.path.insert(0, "/opt/trn_rl_repo")

from contextlib import ExitStack

import ml_dtypes
import numpy as np

import concourse.bass as bass
import concourse.tile as tile
from concourse import bacc, mybir
from concourse.bass_utils import run_bass_kernel_spmd

F32 = mybir.dt.float32
BF16 = mybir.dt.bfloat16
NPBF = ml_dtypes.bfloat16

B, S, H = 2, 2048, 2048
NH, HD = 16, 128
NCORES = 8
NH_LOC = NH // NCORES          # 2 heads per core
T = B * S                      # 4096 tokens
TT = 512                       # token tile
NT = T // TT                   # 8 token tiles
KC = H // 128                  # 16 contraction chunks
SHARD = T // NCORES            # 512 tokens per core output shard
INV_SQRT_D = 1.0 / float(np.sqrt(HD))

_CACHE = {}


def build_graph():
    nc = bacc.Bacc("TRN2", target_bir_lowering=False, debug=False,
                   num_devices=NCORES)

    hiddenT = nc.dram_tensor("hiddenT", [H, T], BF16, kind="ExternalInput")
    cosT = nc.dram_tensor("cosT", [HD, S], F32, kind="ExternalInput")
    nsinT = nc.dram_tensor("nsinT", [HD, S], F32, kind="ExternalInput")
    wqT = nc.dram_tensor("wqT", [H, NH_LOC * HD], BF16, kind="ExternalInput")
    wkT = nc.dram_tensor("wkT", [H, NH_LOC * HD], BF16, kind="ExternalInput")
    wvT = nc.dram_tensor("wvT", [H, NH_LOC * HD], BF16, kind="ExternalInput")
    woT = nc.dram_tensor("woT", [H, H], BF16, kind="ExternalInput")
    out = nc.dram_tensor("out", [H, SHARD], F32, kind="ExternalOutput")

    with tile.TileContext(nc) as tc:
        with ExitStack() as big:
            const = big.enter_context(tc.tile_pool(name="const", bufs=1))
            ones_k = const.tile([128, 1], BF16, tag="ones_k")
            nc.any.memset(ones_k[:], 1.0)
            ones_m = const.tile([1, 128], BF16, tag="ones_m")
            nc.any.memset(ones_m[:], 1.0)

            cs_pool = big.enter_context(tc.tile_pool(name="cs", bufs=1))
            cos_t = cs_pool.tile([HD, S], F32, tag="cos")
            nsin_t = cs_pool.tile([HD, S], F32, tag="nsin")

            # ---- long-lived activation pools (freed before final proj) ----
            act_stack = ExitStack()
            wpool = act_stack.enter_context(tc.tile_pool(name="w", bufs=3 * KC))
            w_t = {}
            for name, src in (("q", wqT), ("k", wkT), ("v", wvT)):
                for f in range(KC):
                    t = wpool.tile([128, NH_LOC * HD], BF16, tag="w",
                                   name=f"w_{name}_{f}")
                    eng = nc.sync if name == "q" else nc.gpsimd
                    eng.dma_start(t[:], src[128 * f:128 * (f + 1), :])
                    w_t[(name, f)] = t
            nc.gpsimd.dma_start(cos_t[:], cosT[:, :])
            nc.gpsimd.dma_start(nsin_t[:], nsinT[:, :])

            qk_pool = act_stack.enter_context(tc.tile_pool(name="qk", bufs=4 * NT))
            v_pool = act_stack.enter_context(tc.tile_pool(name="v", bufs=T // 128))

            qk_t = {}   # (qk, head, ttile) -> [128, TT] sbuf bf16
            v_t = []    # t-chunk -> [128, NH_LOC*HD] sbuf bf16

            # ---------------- phase 1: projections + RoPE ----------------
            with ExitStack() as ph1:
                ht_pool = ph1.enter_context(tc.tile_pool(name="ht", bufs=KC + 2))
                psqk = ph1.enter_context(
                    tc.tile_pool(name="psqk", bufs=5, space="PSUM"))
                psv = ph1.enter_context(
                    tc.tile_pool(name="psv", bufs=3, space="PSUM"))
                tqp = ph1.enter_context(tc.tile_pool(name="tqp", bufs=2))
                rotp = ph1.enter_context(tc.tile_pool(name="rotp", bufs=2))

                for tt in range(NT):
                    t0 = tt * TT
                    i0 = t0 % S  # position within batch (cos/sin index)
                    ht = [ht_pool.tile([128, TT], BF16, tag="ht",
                                       name=f"ht{tt}_{i}")
                          for i in range(KC)]
                    for f in range(KC):
                        nc.sync.dma_start(
                            ht[f][:], hiddenT[128 * f:128 * (f + 1),
                                              t0:t0 + TT])
                    # q/k projections per head -> PSUM [128=HD, TT]
                    for name in ("q", "k"):
                        for h in range(NH_LOC):
                            ps = psqk.tile([128, TT], F32, tag="psqk")
                            for f in range(KC):
                                nc.tensor.matmul(
                                    ps[:],
                                    w_t[(name, f)][:, 128 * h:128 * (h + 1)],
                                    ht[f][:],
                                    start=(f == 0), stop=(f == KC - 1))
                            # RoPE: rot = shifted halves * nsin; out = x*cos + rot
                            tq = tqp.tile([128, TT], F32, tag="tq")
                            nc.vector.scalar_tensor_tensor(
                                tq[:], ps[:], 0.0, cos_t[:, i0:i0 + TT],
                                op0=mybir.AluOpType.bypass,
                                op1=mybir.AluOpType.mult)
                            rot = rotp.tile([128, TT], F32, tag="rot")
                            nc.vector.scalar_tensor_tensor(
                                rot[0:64, :], ps[64:128, :], 0.0,
                                nsin_t[0:64, i0:i0 + TT],
                                op0=mybir.AluOpType.bypass,
                                op1=mybir.AluOpType.mult)
                            nc.vector.scalar_tensor_tensor(
                                rot[64:128, :], ps[0:64, :], 0.0,
                                nsin_t[64:128, i0:i0 + TT],
                                op0=mybir.AluOpType.bypass,
                                op1=mybir.AluOpType.mult)
                            dst = qk_pool.tile([128, TT], BF16, tag="qk")
                            nc.vector.scalar_tensor_tensor(
                                dst[:], tq[:], 0.0, rot[:],
                                op0=mybir.AluOpType.bypass,
                                op1=mybir.AluOpType.add)
                            qk_t[(name, h, tt)] = dst
                    # v natural layout: [t128, 256] both heads
                    for sub in range(TT // 128):
                        ps = psv.tile([128, NH_LOC * HD], F32, tag="psv")
                        for f in range(KC):
                            nc.tensor.matmul(
                                ps[:],
                                ht[f][:, 128 * sub:128 * (sub + 1)],
                                w_t[("v", f)][:],
                                start=(f == 0), stop=(f == KC - 1))
                        vt = v_pool.tile([128, NH_LOC * HD], BF16, tag="v")
                        nc.scalar.copy(vt[:], ps[:])
                        v_t.append(vt)

            # --------- phase 2: attention + A2A bounce-in writes ---------
            dram = big.enter_context(tc.tile_pool(name="dram", bufs=1,
                                                  space="DRAM"))
            bi_h = [dram.tile([NCORES * 128, SHARD], BF16, tag=f"bi{h}",
                              name=f"bi{h}") for h in range(NH_LOC)]
            bo_h = [dram.tile([NCORES * 128, SHARD], BF16, tag=f"bo{h}",
                              name=f"bo{h}") for h in range(NH_LOC)]

            with ExitStack() as ph2:
                stp = ph2.enter_context(
                    tc.tile_pool(name="stp", bufs=5, space="PSUM"))
                accp = ph2.enter_context(
                    tc.tile_pool(name="accp", bufs=2, space="PSUM"))
                rsp = ph2.enter_context(
                    tc.tile_pool(name="rsp", bufs=1, space="PSUM"))
                ptp = ph2.enter_context(tc.tile_pool(name="ptp", bufs=4))
                smallp = ph2.enter_context(tc.tile_pool(name="smallp", bufs=4))
                rbsb = ph2.enter_context(tc.tile_pool(name="rbsb", bufs=2))
                ofp = ph2.enter_context(tc.tile_pool(name="ofp", bufs=3))

                for h in range(NH_LOC):
                    for b in range(B):
                        for ib in range(S // TT):    # i-block within batch
                            q_tile = qk_t[("q", h, 4 * b + ib)]
                            acc = accp.tile([128, TT], F32, tag="accp",
                                            name="acc")
                            rs = rsp.tile([1, TT], F32, tag="rsp", name="rs")
                            sts = [None] * 16

                            def emit_st(j):
                                kt = qk_t[("k", h, 4 * b + j // 4)]
                                co = 128 * (j % 4)
                                sts[j] = stp.tile([128, TT], F32, tag="stp",
                                                  name="st")
                                nc.tensor.matmul(
                                    sts[j][:], kt[:, co:co + 128],
                                    q_tile[:], start=True, stop=True)

                            emit_st(0)
                            emit_st(1)
                            emit_st(2)
                            for jt in range(S // 128):   # j chunks of 128
                                if jt + 3 < 16:
                                    emit_st(jt + 3)
                                pt = ptp.tile([128, TT], BF16, tag="ptp",
                                              name="pt")
                                nc.scalar.activation(
                                    pt[:], sts[jt][:],
                                    mybir.ActivationFunctionType.Exp,
                                    scale=INV_SQRT_D)
                                nc.tensor.matmul(
                                    acc[:],
                                    v_t[16 * b + jt][:, 128 * h:128 * (h + 1)],
                                    pt[:],
                                    start=(jt == 0), stop=(jt == 15))
                                nc.tensor.matmul(
                                    rs[:], ones_k[:], pt[:],
                                    start=(jt == 0), stop=(jt == 15))
                            # softmax epilogue: no PE ops (gpsimd bcast)
                            rs_sb = smallp.tile([1, TT], F32, tag="rs_sb",
                                                name="rs_sb")
                            nc.scalar.copy(rs_sb[:], rs[:])
                            rec = smallp.tile([1, TT], F32, tag="rec",
                                              name="rec")
                            nc.vector.reciprocal_approx_fast(rec[:], rs_sb[:])
                            rsb = rbsb.tile([128, TT], F32, tag="rb_sb",
                                            name="rsb")
                            nc.gpsimd.partition_broadcast(rsb[:], rec[:])
                            of = ofp.tile([128, TT], BF16, tag="of", name="of")
                            nc.vector.scalar_tensor_tensor(
                                of[:], acc[:], 0.0, rsb[:],
                                op0=mybir.AluOpType.bypass,
                                op1=mybir.AluOpType.mult)
                            row0 = 128 * (4 * b + ib)
                            nc.sync.dma_start(
                                bi_h[h][row0:row0 + 128, :], of[:])
                    if h == 0:
                        nc.gpsimd.collective_compute(
                            "AllToAll", mybir.AluOpType.bypass,
                            replica_groups=[list(range(NCORES))],
                            ins=[bi_h[0][:].opt()], outs=[bo_h[0][:].opt()])

            act_stack.close()   # free qk/v/w SBUF before final phase

            # ---------- phase 3: second AllToAll + output projection ----------
            nc.gpsimd.collective_compute(
                "AllToAll", mybir.AluOpType.bypass,
                replica_groups=[list(range(NCORES))],
                ins=[bi_h[1][:].opt()], outs=[bo_h[1][:].opt()])

            with ExitStack() as ph3:
                ogp = ph3.enter_context(tc.tile_pool(name="ogp", bufs=KC))
                wop = ph3.enter_context(tc.tile_pool(name="wop", bufs=12))
                yps = ph3.enter_context(
                    tc.tile_pool(name="yps", bufs=4, space="PSUM"))
                ysb = ph3.enter_context(tc.tile_pool(name="ysb", bufs=4))

                og = []
                for f in range(KC):     # f = 2r + h -> bo_h[h] rows 128r
                    t = ogp.tile([128, SHARD], BF16, tag="og", name=f"og{f}")
                    r, hh = f // 2, f % 2
                    nc.sync.dma_start(
                        t[:], bo_h[hh][128 * r:128 * (r + 1), :])
                    og.append(t)
                for gg in range(4):              # groups of 4 g-tiles
                    wo_g = []
                    for f in range(KC):
                        t = wop.tile([128, 512], BF16, tag="wo",
                                     name=f"wo{gg}_{f}")
                        nc.sync.dma_start(
                            t[:], woT[128 * f:128 * (f + 1),
                                      512 * gg:512 * (gg + 1)])
                        wo_g.append(t)
                    for gi in range(4):
                        g = 4 * gg + gi
                        yp = yps.tile([128, SHARD], F32, tag="yps", name="yp")
                        for fi, f in enumerate(
                                [x for x in range(KC) if x % 2 == 0]
                                + [x for x in range(KC) if x % 2 == 1]):
                            nc.tensor.matmul(
                                yp[:],
                                wo_g[f][:, 128 * gi:128 * (gi + 1)],
                                og[f][:],
                                start=(fi == 0), stop=(fi == KC - 1))
                        ys = ysb.tile([128, SHARD], F32, tag="ysb", name="ys")
                        nc.scalar.copy(ys[:], yp[:])
                        nc.sync.dma_start(out[128 * g:128 * (g + 1), :], ys[:])

    nc.compile()
    return nc


def _prep_inputs(hidden, cos, sin, Wq, Wk, Wv, Wo):
    hf = np.ascontiguousarray(hidden.reshape(T, H).T.astype(NPBF))
    cosT = np.ascontiguousarray(cos.T).astype(np.float32)
    nsinT = np.ascontiguousarray(sin.T).astype(np.float32)
    nsinT[0:HD // 2] *= -1.0
    woT = np.ascontiguousarray(Wo.T.astype(NPBF))
    in_maps = []
    for c in range(NCORES):
        r0, r1 = 256 * c, 256 * (c + 1)
        in_maps.append({
            "hiddenT": hf,
            "cosT": cosT,
            "nsinT": nsinT,
            "wqT": np.ascontiguousarray(Wq[r0:r1].T.astype(NPBF)),
            "wkT": np.ascontiguousarray(Wk[r0:r1].T.astype(NPBF)),
            "wvT": np.ascontiguousarray(Wv[r0:r1].T.astype(NPBF)),
            "woT": woT,
        })
    return in_maps


def kernel(hidden, cos, sin, attention_mask, Wq, Wk, Wv, Wo, **run_kwargs):
    if "nc" not in _CACHE:
        _CACHE["nc"] = build_graph()
    nc = _CACHE["nc"]
    in_maps = _prep_inputs(hidden, cos, sin, Wq, Wk, Wv, Wo)
    res = run_bass_kernel_spmd(nc, in_maps, core_ids=list(range(NCORES)),
                               **run_kwargs)
    _CACHE["last_result"] = res
    outs = res.results if hasattr(res, "results") else res
    y = np.empty((T, H), dtype=np.float32)
    for c in range(NCORES):
        y[SHARD * c:SHARD * (c + 1), :] = outs[c]["out"].T
    return y.reshape(B, S, H)
